# revision 36
# baseline (speedup 1.0000x reference)
"""ErrorAwareEdgeLoss Trainium2 kernel.

Math: loss = mean_b [ (sum_e w_be * P[b,i_e,:] @ D @ P[b,j_e,:]) / max(sum_e w_be, 1e-8) ]

Reformulation:
    G_b = (P_b @ D) @ P_b^T            (two 256^3 matmuls on the PE, bf16)
    sum_e w_be * cost_e = <W_b, G_b>   with W_b[i,j] = sum of w over edges (i,j)

W_b is built entirely in SBUF with one gpsimd local_scatter per batch:
G_b's element f = 256*i + j lives at partition p = (f>>8)&127, local slot
m = (f>>15)*256 + (f&255) — a bijection, so the host can bin each edge to
its home partition (pure index/layout prep). Exact duplicate edges (same
(i,j)) share a cell; their weights are summed ON DEVICE (a tiny reduce
over a host-laid [cell, copy] array) before the scatter. <W, G> is then
one elementwise multiply + row reduce; no DRAM spill, no gather DMA.

Sharding: data-parallel over batch: 8 NeuronCores x 8 batches. Each core
emits per-sample partial sums (sum w*g and sum w per batch); the host
performs the final divide + mean (the all-reduce of the sharding hint).
"""

from contextlib import ExitStack

import ml_dtypes
import numpy as np

import concourse.bacc as bacc
import concourse.bass as bass
import concourse.mybir as mybir
import concourse.tile as tile
from concourse.bass_utils import run_bass_kernel_spmd

B, N, E = 64, 256, 8192
NCORES = 8
BPC = B // NCORES  # batches per core
NPAIR = BPC // 2

C0 = 104  # scatter slots per partition (singles + dup cells)
CD = 16   # dup-cell slots (tail of the C0 range)
CS = C0 - CD  # single-cell slots
R = 12    # max copies per dup cell

f32 = mybir.dt.float32
bf16 = mybir.dt.bfloat16
i16 = mybir.dt.int16

NE = 512  # W/G elements per partition


def _build_bass():
    nc = bacc.Bacc("TRN2", target_bir_lowering=False, debug=False)

    # pt[t, p, kc, b2, i] = P[2t+b2, i, kc*128+p]
    pt_in = nc.dram_tensor("pt", [NPAIR, 128, 2, 2, N], bf16, kind="ExternalInput")
    d_in = nc.dram_tensor("derr", [128, 2, N], bf16, kind="ExternalInput")
    # per-(batch, partition) scatter plan (see _prep_in_maps)
    ei_in = nc.dram_tensor("idx0", [128, BPC, C0], i16, kind="ExternalInput")
    ed_in = nc.dram_tensor("dat0", [128, BPC, C0], bf16, kind="ExternalInput")
    wd_in = nc.dram_tensor("wdup", [128, BPC, CD, R], f32, kind="ExternalInput")
    out = nc.dram_tensor("out", [1, 2 * BPC], f32, kind="ExternalOutput")

    with tile.TileContext(nc) as tc, ExitStack() as ctx:
        const_pool = ctx.enter_context(tc.tile_pool(name="const", bufs=1))
        pt_pool = ctx.enter_context(tc.tile_pool(name="pt", bufs=4))
        qt_pool = ctx.enter_context(tc.tile_pool(name="qt", bufs=3))
        g_pool = ctx.enter_context(tc.tile_pool(name="g", bufs=3))
        w_pool = ctx.enter_context(tc.tile_pool(name="w", bufs=4))
        e_pool = ctx.enter_context(tc.tile_pool(name="e", bufs=4))
        psA_pool = ctx.enter_context(tc.tile_pool(name="psA", bufs=4, space="PSUM"))
        psB_pool = ctx.enter_context(tc.tile_pool(name="psB", bufs=3, space="PSUM"))
        psum1_pool = ctx.enter_context(tc.tile_pool(name="ps1", bufs=1, space="PSUM"))

        # inputs (sync queue; d + pt0 first, edge plan next, later pts after)
        d_sb = const_pool.tile([128, 2, N], bf16)
        nc.sync.dma_start(d_sb[:], d_in[:])
        idx0_sb = const_pool.tile([128, BPC, C0], i16)
        dat0_sb = const_pool.tile([128, BPC, C0], bf16)
        wdup_sb = const_pool.tile([128, BPC, CD, R], f32)
        # per-batch partials: cols [0,BPC) = sum(w*g), cols [BPC,2*BPC) = sum(w)
        red_sb = const_pool.tile([128, 2 * BPC], f32)
        ones_sb = const_pool.tile([128, 1], f32)
        nc.vector.memset(ones_sb[:], 1.0)

        pending = None  # (w_tile, g_psum, b) awaiting <W, G> — one batch behind

        def flush_pending():
            nonlocal pending
            if pending is None:
                return
            w_sb, g_sb, b = pending
            prod = e_pool.tile([128, NE], bf16, tag="prod")
            nc.vector.tensor_tensor(
                out=prod[:],
                in0=w_sb[:],
                in1=g_sb[:].rearrange("p a b -> p (a b)"),
                op=mybir.AluOpType.mult,
            )
            nc.vector.tensor_reduce(
                out=red_sb[:, b : b + 1],
                in_=prod[:],
                axis=mybir.AxisListType.X,
                op=mybir.AluOpType.add,
            )
            pending = None

        pt_tiles = [None] * NPAIR
        qt_tiles = [None] * NPAIR

        def load_pt(t):
            # kc halves on different queues: the two 256KB transfers run in
            # parallel instead of serializing behind one DMA queue
            pt2 = pt_pool.tile([128, 2, 2, N], bf16)
            nc.sync.dma_start(pt2[:, 0], pt_in[t, :, 0])
            nc.gpsimd.dma_start(pt2[:, 1], pt_in[t, :, 1])
            pt_tiles[t] = pt2

        def do_qt(t):
            # QT[n, (b2, i)] = sum_k D[k, n] * PT[k, (b2, i)]
            qt_sb = qt_pool.tile([128, 2, 2, N], bf16)  # (ncx, b2, i)
            for ncx in range(2):
                qt_ps = psA_pool.tile([128, 2, N], f32, tag="qtps")
                for kc in range(2):
                    nc.tensor.matmul(
                        qt_ps[:].rearrange("p a b -> p (a b)"),
                        lhsT=d_sb[:, kc, ncx * 128 : (ncx + 1) * 128],
                        rhs=pt_tiles[t][:, kc, :, :].rearrange("p a b -> p (a b)"),
                        start=(kc == 0),
                        stop=(kc == 1),
                    )
                nc.scalar.copy(qt_sb[:, ncx], qt_ps[:])
            qt_tiles[t] = qt_sb

        load_pt(0)
        nc.scalar.dma_start(idx0_sb[:], ei_in[:])
        nc.scalar.dma_start(dat0_sb[:], ed_in[:])
        nc.scalar.dma_start(wdup_sb[:], wd_in[:])
        # hoisted batch-level prep: merge dup weights into dat0 and
        # compute all sum(w) columns in three wide ops
        ws_all = const_pool.tile([128, BPC, CD], f32)
        nc.vector.tensor_reduce(
            out=ws_all[:], in_=wdup_sb[:],
            axis=mybir.AxisListType.X, op=mybir.AluOpType.add,
        )
        nc.vector.tensor_copy(dat0_sb[:, :, CS:], ws_all[:])
        nc.vector.tensor_reduce(
            out=red_sb[:, BPC:], in_=dat0_sb[:],
            axis=mybir.AxisListType.X, op=mybir.AluOpType.add,
        )
        do_qt(0)
        load_pt(1)

        for t in range(NPAIR):
            # software pipeline: QT one pair ahead keeps the PE dense
            if t + 1 < NPAIR:
                do_qt(t + 1)
                if t + 2 < NPAIR:
                    load_pt(t + 2)
            qt_sb = qt_tiles[t]
            pt2 = pt_tiles[t]

            for b2 in range(2):
                b = 2 * t + b2
                # ---- W scatter (dup weights already merged into dat0)
                w_sb = w_pool.tile([128, NE], bf16)
                nc.gpsimd.local_scatter(
                    out_ap=w_sb[:],
                    data_ap=dat0_sb[:, b],
                    idxs_ap=idx0_sb[:, b],
                    channels=128,
                    num_elems=NE,
                    num_idxs=C0,
                )
                # ---- G[(ic), j] = sum_n QT[n, i] * PT[n, j]
                g_ps = psB_pool.tile([128, 2, N], f32, tag="gps")  # (ic, j)
                for ic in range(2):
                    for ncx in range(2):
                        nc.tensor.matmul(
                            g_ps[:, ic, :],
                            lhsT=qt_sb[:, ncx, b2, ic * 128 : (ic + 1) * 128],
                            rhs=pt2[:, ncx, b2, :],
                            start=(ncx == 0),
                            stop=(ncx == 1),
                        )

                # ---- <W, G>: reduce the PREVIOUS batch first (pipelined);
                # the multiply reads G straight from PSUM (no copy)
                flush_pending()
                pending = (w_sb, g_ps, b)

        flush_pending()

        # ---- cross-partition reduce of all partials in one matmul
        red_ps = psum1_pool.tile([1, 2 * BPC], f32, tag="redps")
        nc.tensor.matmul(
            red_ps[:], lhsT=ones_sb[:], rhs=red_sb[:], start=True, stop=True
        )
        fin = const_pool.tile([1, 2 * BPC], f32)
        nc.vector.tensor_copy(fin[:], red_ps[:])
        nc.sync.dma_start(out[:], fin[:])

    if not nc.is_finalized():
        nc.finalize()
    return nc


_NC_CACHE = {}


def _get_nc():
    if "nc" not in _NC_CACHE:
        _NC_CACHE["nc"] = _build_bass()
    return _NC_CACHE["nc"]


def _prep_in_maps(P, d_error, edge_i, edge_j, edge_w):
    P = np.asarray(P, dtype=np.float32)
    d_error = np.asarray(d_error, dtype=np.float32)
    edge_i = np.asarray(edge_i, dtype=np.int64)
    edge_j = np.asarray(edge_j, dtype=np.int64)
    edge_w = np.asarray(edge_w, dtype=np.float32)

    # P^T pairs: pt[t, p, kc, b2, i] = P[2t+b2, i, kc*128+p]
    PT = np.ascontiguousarray(np.transpose(P, (0, 2, 1)))  # [B, N(k), N(i)]
    PT = PT.reshape(B // 2, 2, 2, 128, N).transpose(0, 3, 2, 1, 4)
    PT = np.ascontiguousarray(PT).astype(ml_dtypes.bfloat16)
    D = np.ascontiguousarray(d_error.reshape(2, 128, N).transpose(1, 0, 2))
    D = D.astype(ml_dtypes.bfloat16)

    # scatter plan: edge f = 256*i + j -> partition p = (f>>8)&127,
    # slot m = (f>>15)*256 + (f&255). Exact-duplicate edges share a cell.
    f_all = (edge_i << 8) | edge_j  # [B, E]
    p_all = (f_all >> 8) & 127
    m_all = ((f_all >> 15) << 8) | (f_all & 255)

    idx0 = np.full((B, 128, C0), -1, np.int16)
    dat0 = np.zeros((B, 128, C0), np.float32)
    wdup = np.zeros((B, 128, CD, R), np.float32)

    for b in range(B):
        pb, mb, wb = p_all[b], m_all[b], edge_w[b]
        order = np.lexsort((mb, pb))
        ps, ms, ws = pb[order], mb[order], wb[order]
        first = np.ones(E, bool)
        first[1:] = (ps[1:] != ps[:-1]) | (ms[1:] != ms[:-1])
        fidx = np.flatnonzero(first)
        ccnt = np.diff(np.append(fidx, E))
        cp, cm = ps[fidx], ms[fidx]
        # per-partition sequential numbering of single / dup cells
        single = ccnt == 1
        for pp in np.unique(cp):
            sel = cp == pp
            cells_m = cm[sel]
            cells_n = ccnt[sel]
            cells_start = fidx[sel]
            s_slot = 0
            d_slot = 0
            for cmi, cni, csi in zip(cells_m, cells_n, cells_start):
                if cni == 1:
                    assert s_slot < CS
                    idx0[b, pp, s_slot] = cmi
                    dat0[b, pp, s_slot] = ws[csi]
                    s_slot += 1
                else:
                    assert d_slot < CD and cni <= R
                    idx0[b, pp, CS + d_slot] = cmi
                    wdup[b, pp, d_slot, :cni] = ws[csi : csi + cni]
                    d_slot += 1

    dat0 = dat0.astype(ml_dtypes.bfloat16)
    # -> [p, b, ...]
    idx0 = np.ascontiguousarray(idx0.transpose(1, 0, 2))
    dat0 = np.ascontiguousarray(dat0.transpose(1, 0, 2))
    wdup = np.ascontiguousarray(wdup.transpose(1, 0, 2, 3))

    in_maps = []
    for c in range(NCORES):
        sl = slice(c * BPC, (c + 1) * BPC)
        in_maps.append(
            {
                "pt": np.ascontiguousarray(PT[c * NPAIR : (c + 1) * NPAIR]),
                "derr": D,
                "idx0": np.ascontiguousarray(idx0[:, sl]),
                "dat0": np.ascontiguousarray(dat0[:, sl]),
                "wdup": np.ascontiguousarray(wdup[:, sl]),
            }
        )
    return in_maps


def run(P, d_error, edge_i, edge_j, edge_w, trace=False):
    """Run on 8 cores; returns (loss_scalar, BassKernelResults)."""
    nc = _get_nc()
    in_maps = _prep_in_maps(P, d_error, edge_i, edge_j, edge_w)
    res = run_bass_kernel_spmd(
        nc, in_maps, core_ids=list(range(NCORES)), trace=trace
    )
    # host-side all-reduce: loss = mean_b( sl_b / max(sw_b, 1e-8) )
    acc = 0.0
    for r in res.results:
        part = np.asarray(r["out"], dtype=np.float64).reshape(2 * BPC)
        sl, sw = part[:BPC], part[BPC:]
        acc += float(np.sum(sl / np.maximum(sw, 1e-8)))
    loss = np.float32(acc / B)
    return loss, res


def kernel(P, d_error, edge_i, edge_j, edge_w):
    loss, _ = run(P, d_error, edge_i, edge_j, edge_w, trace=False)
    return np.asarray(loss, dtype=np.float32)


# revision 37
# speedup vs baseline: 1.0363x; 1.0363x over previous
"""ErrorAwareEdgeLoss Trainium2 kernel.

Math: loss = mean_b [ (sum_e w_be * P[b,i_e,:] @ D @ P[b,j_e,:]) / max(sum_e w_be, 1e-8) ]

Reformulation:
    G_b = (P_b @ D) @ P_b^T            (two 256^3 matmuls on the PE, bf16)
    sum_e w_be * cost_e = <W_b, G_b>   with W_b[i,j] = sum of w over edges (i,j)

W_b is built entirely in SBUF with one gpsimd local_scatter per batch:
G_b's element f = 256*i + j lives at partition p = (f>>8)&127, local slot
m = (f>>15)*256 + (f&255) — a bijection, so the host can bin each edge to
its home partition (pure index/layout prep). Exact duplicate edges (same
(i,j)) share a cell; their weights are summed ON DEVICE (a tiny reduce
over a host-laid [cell, copy] array) before the scatter. <W, G> is then
one elementwise multiply + row reduce; no DRAM spill, no gather DMA.

Sharding: data-parallel over batch: 8 NeuronCores x 8 batches. Each core
emits per-sample partial sums (sum w*g and sum w per batch); the host
performs the final divide + mean (the all-reduce of the sharding hint).
"""

from contextlib import ExitStack

import ml_dtypes
import numpy as np

import concourse.bacc as bacc
import concourse.bass as bass
import concourse.mybir as mybir
import concourse.tile as tile
from concourse.bass_utils import run_bass_kernel_spmd

B, N, E = 64, 256, 8192
NCORES = 8
BPC = B // NCORES  # batches per core
NPAIR = BPC // 2

C0 = 104  # scatter slots per partition (singles + dup cells)
CD = 16   # dup-cell slots (tail of the C0 range)
CS = C0 - CD  # single-cell slots
R = 12    # max copies per dup cell

f32 = mybir.dt.float32
bf16 = mybir.dt.bfloat16
i16 = mybir.dt.int16

NE = 512  # W/G elements per partition


def _build_bass():
    nc = bacc.Bacc("TRN2", target_bir_lowering=False, debug=False)

    # pt[t, p, kc, b2, i] = P[2t+b2, i, kc*128+p]
    pt_in = nc.dram_tensor("pt", [NPAIR, 128, 2, 2, N], bf16, kind="ExternalInput")
    d_in = nc.dram_tensor("derr", [128, 2, N], bf16, kind="ExternalInput")
    # per-(batch, partition) scatter plan (see _prep_in_maps)
    ei_in = nc.dram_tensor("idx0", [128, BPC, C0], i16, kind="ExternalInput")
    ed_in = nc.dram_tensor("dat0", [128, BPC, C0], bf16, kind="ExternalInput")
    wd_in = nc.dram_tensor("wdup", [128, BPC, CD, R], f32, kind="ExternalInput")
    out = nc.dram_tensor("out", [1, 2 * BPC], f32, kind="ExternalOutput")

    with tile.TileContext(nc) as tc, ExitStack() as ctx:
        const_pool = ctx.enter_context(tc.tile_pool(name="const", bufs=1))
        pt_pool = ctx.enter_context(tc.tile_pool(name="pt", bufs=4))
        qt_pool = ctx.enter_context(tc.tile_pool(name="qt", bufs=3))
        g_pool = ctx.enter_context(tc.tile_pool(name="g", bufs=3))
        w_pool = ctx.enter_context(tc.tile_pool(name="w", bufs=4))
        e_pool = ctx.enter_context(tc.tile_pool(name="e", bufs=4))
        psA_pool = ctx.enter_context(tc.tile_pool(name="psA", bufs=4, space="PSUM"))
        psB_pool = ctx.enter_context(tc.tile_pool(name="psB", bufs=3, space="PSUM"))
        psum1_pool = ctx.enter_context(tc.tile_pool(name="ps1", bufs=1, space="PSUM"))

        # inputs (sync queue; d + pt0 first, edge plan next, later pts after)
        d_sb = const_pool.tile([128, 2, N], bf16)
        nc.sync.dma_start(d_sb[:], d_in[:])
        idx0_sb = const_pool.tile([128, BPC, C0], i16)
        dat0_sb = const_pool.tile([128, BPC, C0], bf16)
        wdup_sb = const_pool.tile([128, BPC, CD, R], f32)
        # per-batch partials: cols [0,BPC) = sum(w*g), cols [BPC,2*BPC) = sum(w)
        red_sb = const_pool.tile([128, 2 * BPC], f32)
        ones_sb = const_pool.tile([128, 1], f32)
        nc.vector.memset(ones_sb[:], 1.0)

        pending = None  # (w_tile, g_psum, b) awaiting <W, G> — one batch behind

        def flush_pending():
            nonlocal pending
            if pending is None:
                return
            w_sb, g_sb, b = pending
            prod = e_pool.tile([128, NE], bf16, tag="prod")
            nc.vector.tensor_tensor(
                out=prod[:],
                in0=w_sb[:],
                in1=g_sb[:].rearrange("p a b -> p (a b)"),
                op=mybir.AluOpType.mult,
            )
            nc.vector.tensor_reduce(
                out=red_sb[:, b : b + 1],
                in_=prod[:],
                axis=mybir.AxisListType.X,
                op=mybir.AluOpType.add,
            )
            pending = None

        pt_tiles = [None] * NPAIR
        qt_tiles = [None] * NPAIR

        def load_pt(t):
            pt2 = pt_pool.tile([128, 2, 2, N], bf16)
            if t == 0:
                # kc halves on different queues: the two 256KB transfers run
                # in parallel, pulling the first matmul earlier
                nc.sync.dma_start(pt2[:, 0], pt_in[t, :, 0])
                nc.scalar.dma_start(pt2[:, 1], pt_in[t, :, 1])
            else:
                nc.sync.dma_start(pt2[:], pt_in[t])
            pt_tiles[t] = pt2

        def do_qt(t):
            # QT[n, (b2, i)] = sum_k D[k, n] * PT[k, (b2, i)]
            qt_sb = qt_pool.tile([128, 2, 2, N], bf16)  # (ncx, b2, i)
            for ncx in range(2):
                qt_ps = psA_pool.tile([128, 2, N], f32, tag="qtps")
                for kc in range(2):
                    nc.tensor.matmul(
                        qt_ps[:].rearrange("p a b -> p (a b)"),
                        lhsT=d_sb[:, kc, ncx * 128 : (ncx + 1) * 128],
                        rhs=pt_tiles[t][:, kc, :, :].rearrange("p a b -> p (a b)"),
                        start=(kc == 0),
                        stop=(kc == 1),
                    )
                nc.scalar.copy(qt_sb[:, ncx], qt_ps[:])
            qt_tiles[t] = qt_sb

        load_pt(0)
        nc.scalar.dma_start(idx0_sb[:], ei_in[:])
        nc.scalar.dma_start(dat0_sb[:], ed_in[:])
        nc.scalar.dma_start(wdup_sb[:], wd_in[:])
        # hoisted batch-level prep: merge dup weights into dat0 and
        # compute all sum(w) columns in three wide ops
        ws_all = const_pool.tile([128, BPC, CD], f32)
        nc.vector.tensor_reduce(
            out=ws_all[:], in_=wdup_sb[:],
            axis=mybir.AxisListType.X, op=mybir.AluOpType.add,
        )
        nc.vector.tensor_copy(dat0_sb[:, :, CS:], ws_all[:])
        nc.vector.tensor_reduce(
            out=red_sb[:, BPC:], in_=dat0_sb[:],
            axis=mybir.AxisListType.X, op=mybir.AluOpType.add,
        )
        do_qt(0)
        load_pt(1)

        for t in range(NPAIR):
            # software pipeline: QT one pair ahead keeps the PE dense
            if t + 1 < NPAIR:
                do_qt(t + 1)
                if t + 2 < NPAIR:
                    load_pt(t + 2)
            qt_sb = qt_tiles[t]
            pt2 = pt_tiles[t]

            for b2 in range(2):
                b = 2 * t + b2
                # ---- W scatter (dup weights already merged into dat0)
                w_sb = w_pool.tile([128, NE], bf16)
                nc.gpsimd.local_scatter(
                    out_ap=w_sb[:],
                    data_ap=dat0_sb[:, b],
                    idxs_ap=idx0_sb[:, b],
                    channels=128,
                    num_elems=NE,
                    num_idxs=C0,
                )
                # ---- G[(ic), j] = sum_n QT[n, i] * PT[n, j]
                g_ps = psB_pool.tile([128, 2, N], f32, tag="gps")  # (ic, j)
                for ic in range(2):
                    for ncx in range(2):
                        nc.tensor.matmul(
                            g_ps[:, ic, :],
                            lhsT=qt_sb[:, ncx, b2, ic * 128 : (ic + 1) * 128],
                            rhs=pt2[:, ncx, b2, :],
                            start=(ncx == 0),
                            stop=(ncx == 1),
                        )

                # ---- <W, G>: reduce the PREVIOUS batch first (pipelined);
                # the multiply reads G straight from PSUM (no copy)
                flush_pending()
                pending = (w_sb, g_ps, b)

        flush_pending()

        # ---- cross-partition reduce of all partials in one matmul
        red_ps = psum1_pool.tile([1, 2 * BPC], f32, tag="redps")
        nc.tensor.matmul(
            red_ps[:], lhsT=ones_sb[:], rhs=red_sb[:], start=True, stop=True
        )
        fin = const_pool.tile([1, 2 * BPC], f32)
        nc.vector.tensor_copy(fin[:], red_ps[:])
        nc.sync.dma_start(out[:], fin[:])

    if not nc.is_finalized():
        nc.finalize()
    return nc


_NC_CACHE = {}


def _get_nc():
    if "nc" not in _NC_CACHE:
        _NC_CACHE["nc"] = _build_bass()
    return _NC_CACHE["nc"]


def _prep_in_maps(P, d_error, edge_i, edge_j, edge_w):
    P = np.asarray(P, dtype=np.float32)
    d_error = np.asarray(d_error, dtype=np.float32)
    edge_i = np.asarray(edge_i, dtype=np.int64)
    edge_j = np.asarray(edge_j, dtype=np.int64)
    edge_w = np.asarray(edge_w, dtype=np.float32)

    # P^T pairs: pt[t, p, kc, b2, i] = P[2t+b2, i, kc*128+p]
    PT = np.ascontiguousarray(np.transpose(P, (0, 2, 1)))  # [B, N(k), N(i)]
    PT = PT.reshape(B // 2, 2, 2, 128, N).transpose(0, 3, 2, 1, 4)
    PT = np.ascontiguousarray(PT).astype(ml_dtypes.bfloat16)
    D = np.ascontiguousarray(d_error.reshape(2, 128, N).transpose(1, 0, 2))
    D = D.astype(ml_dtypes.bfloat16)

    # scatter plan: edge f = 256*i + j -> partition p = (f>>8)&127,
    # slot m = (f>>15)*256 + (f&255). Exact-duplicate edges share a cell.
    f_all = (edge_i << 8) | edge_j  # [B, E]
    p_all = (f_all >> 8) & 127
    m_all = ((f_all >> 15) << 8) | (f_all & 255)

    idx0 = np.full((B, 128, C0), -1, np.int16)
    dat0 = np.zeros((B, 128, C0), np.float32)
    wdup = np.zeros((B, 128, CD, R), np.float32)

    for b in range(B):
        pb, mb, wb = p_all[b], m_all[b], edge_w[b]
        order = np.lexsort((mb, pb))
        ps, ms, ws = pb[order], mb[order], wb[order]
        first = np.ones(E, bool)
        first[1:] = (ps[1:] != ps[:-1]) | (ms[1:] != ms[:-1])
        fidx = np.flatnonzero(first)
        ccnt = np.diff(np.append(fidx, E))
        cp, cm = ps[fidx], ms[fidx]
        # per-partition sequential numbering of single / dup cells
        single = ccnt == 1
        for pp in np.unique(cp):
            sel = cp == pp
            cells_m = cm[sel]
            cells_n = ccnt[sel]
            cells_start = fidx[sel]
            s_slot = 0
            d_slot = 0
            for cmi, cni, csi in zip(cells_m, cells_n, cells_start):
                if cni == 1:
                    assert s_slot < CS
                    idx0[b, pp, s_slot] = cmi
                    dat0[b, pp, s_slot] = ws[csi]
                    s_slot += 1
                else:
                    assert d_slot < CD and cni <= R
                    idx0[b, pp, CS + d_slot] = cmi
                    wdup[b, pp, d_slot, :cni] = ws[csi : csi + cni]
                    d_slot += 1

    dat0 = dat0.astype(ml_dtypes.bfloat16)
    # -> [p, b, ...]
    idx0 = np.ascontiguousarray(idx0.transpose(1, 0, 2))
    dat0 = np.ascontiguousarray(dat0.transpose(1, 0, 2))
    wdup = np.ascontiguousarray(wdup.transpose(1, 0, 2, 3))

    in_maps = []
    for c in range(NCORES):
        sl = slice(c * BPC, (c + 1) * BPC)
        in_maps.append(
            {
                "pt": np.ascontiguousarray(PT[c * NPAIR : (c + 1) * NPAIR]),
                "derr": D,
                "idx0": np.ascontiguousarray(idx0[:, sl]),
                "dat0": np.ascontiguousarray(dat0[:, sl]),
                "wdup": np.ascontiguousarray(wdup[:, sl]),
            }
        )
    return in_maps


def run(P, d_error, edge_i, edge_j, edge_w, trace=False):
    """Run on 8 cores; returns (loss_scalar, BassKernelResults)."""
    nc = _get_nc()
    in_maps = _prep_in_maps(P, d_error, edge_i, edge_j, edge_w)
    res = run_bass_kernel_spmd(
        nc, in_maps, core_ids=list(range(NCORES)), trace=trace
    )
    # host-side all-reduce: loss = mean_b( sl_b / max(sw_b, 1e-8) )
    acc = 0.0
    for r in res.results:
        part = np.asarray(r["out"], dtype=np.float64).reshape(2 * BPC)
        sl, sw = part[:BPC], part[BPC:]
        acc += float(np.sum(sl / np.maximum(sw, 1e-8)))
    loss = np.float32(acc / B)
    return loss, res


def kernel(P, d_error, edge_i, edge_j, edge_w):
    loss, _ = run(P, d_error, edge_i, edge_j, edge_w, trace=False)
    return np.asarray(loss, dtype=np.float32)


# revision 38
# speedup vs baseline: 1.0482x; 1.0115x over previous
"""ErrorAwareEdgeLoss Trainium2 kernel.

Math: loss = mean_b [ (sum_e w_be * P[b,i_e,:] @ D @ P[b,j_e,:]) / max(sum_e w_be, 1e-8) ]

Reformulation:
    G_b = (P_b @ D) @ P_b^T            (two 256^3 matmuls on the PE, bf16)
    sum_e w_be * cost_e = <W_b, G_b>   with W_b[i,j] = sum of w over edges (i,j)

W_b is built entirely in SBUF with one gpsimd local_scatter per batch:
G_b's element f = 256*i + j lives at partition p = (f>>8)&127, local slot
m = (f>>15)*256 + (f&255) — a bijection, so the host can bin each edge to
its home partition (pure index/layout prep). Exact duplicate edges (same
(i,j)) share a cell; their weights are summed ON DEVICE (a tiny reduce
over a host-laid [cell, copy] array) before the scatter. <W, G> is then
one elementwise multiply + row reduce; no DRAM spill, no gather DMA.

Sharding: data-parallel over batch: 8 NeuronCores x 8 batches. Each core
emits per-sample partial sums (sum w*g and sum w per batch); the host
performs the final divide + mean (the all-reduce of the sharding hint).
"""

from contextlib import ExitStack

import ml_dtypes
import numpy as np

import concourse.bacc as bacc
import concourse.bass as bass
import concourse.mybir as mybir
import concourse.tile as tile
from concourse.bass_utils import run_bass_kernel_spmd

B, N, E = 64, 256, 8192
NCORES = 8
BPC = B // NCORES  # batches per core
NPAIR = BPC // 2

C0 = 104  # scatter slots per partition (singles + dup cells)
CD = 16   # dup-cell slots (tail of the C0 range)
CS = C0 - CD  # single-cell slots
R = 12    # max copies per dup cell

f32 = mybir.dt.float32
bf16 = mybir.dt.bfloat16
i16 = mybir.dt.int16

NE = 512  # W/G elements per partition


def _build_bass():
    nc = bacc.Bacc("TRN2", target_bir_lowering=False, debug=False)

    # pt[t, p, kc, b2, i] = P[2t+b2, i, kc*128+p]
    pt_in = nc.dram_tensor("pt", [NPAIR, 128, 2, 2, N], bf16, kind="ExternalInput")
    d_in = nc.dram_tensor("derr", [128, 2, N], bf16, kind="ExternalInput")
    # per-(batch, partition) scatter plan (see _prep_in_maps)
    ei_in = nc.dram_tensor("idx0", [128, BPC, C0], i16, kind="ExternalInput")
    ed_in = nc.dram_tensor("dat0", [128, BPC, C0], bf16, kind="ExternalInput")
    wd_in = nc.dram_tensor("wdup", [128, BPC, CD, R], f32, kind="ExternalInput")
    out = nc.dram_tensor("out", [1, 2 * BPC], f32, kind="ExternalOutput")

    with tile.TileContext(nc) as tc, ExitStack() as ctx:
        const_pool = ctx.enter_context(tc.tile_pool(name="const", bufs=1))
        pt_pool = ctx.enter_context(tc.tile_pool(name="pt", bufs=4))
        qt_pool = ctx.enter_context(tc.tile_pool(name="qt", bufs=3))
        g_pool = ctx.enter_context(tc.tile_pool(name="g", bufs=3))
        w_pool = ctx.enter_context(tc.tile_pool(name="w", bufs=4))
        e_pool = ctx.enter_context(tc.tile_pool(name="e", bufs=4))
        psA_pool = ctx.enter_context(tc.tile_pool(name="psA", bufs=4, space="PSUM"))
        psB_pool = ctx.enter_context(tc.tile_pool(name="psB", bufs=3, space="PSUM"))
        psum1_pool = ctx.enter_context(tc.tile_pool(name="ps1", bufs=1, space="PSUM"))

        # inputs (sync queue; d + pt0 first, edge plan next, later pts after)
        d_sb = const_pool.tile([128, 2, N], bf16)
        nc.sync.dma_start(d_sb[:], d_in[:])
        idx0_sb = const_pool.tile([128, BPC, C0], i16)
        dat0_sb = const_pool.tile([128, BPC, C0], bf16)
        wdup_sb = const_pool.tile([128, BPC, CD, R], f32)
        # per-batch partials: cols [0,BPC) = sum(w*g), cols [BPC,2*BPC) = sum(w)
        red_sb = const_pool.tile([128, 2 * BPC], f32)
        ones_sb = const_pool.tile([128, 1], f32)
        nc.vector.memset(ones_sb[:], 1.0)

        pending = None  # (w_tile, g_psum, b) awaiting <W, G> — one batch behind

        def flush_pending():
            nonlocal pending
            if pending is None:
                return
            w_sb, g_sb, b = pending
            prod = e_pool.tile([128, NE], bf16, tag="prod")
            nc.vector.tensor_tensor(
                out=prod[:],
                in0=w_sb[:],
                in1=g_sb[:].rearrange("p a b -> p (a b)"),
                op=mybir.AluOpType.mult,
            )
            nc.vector.tensor_reduce(
                out=red_sb[:, b : b + 1],
                in_=prod[:],
                axis=mybir.AxisListType.X,
                op=mybir.AluOpType.add,
            )
            pending = None

        pt_tiles = [None] * NPAIR
        qt_tiles = [None] * NPAIR

        def load_pt(t):
            pt2 = pt_pool.tile([128, 2, 2, N], bf16)
            nc.sync.dma_start(pt2[:], pt_in[t])
            pt_tiles[t] = pt2

        def do_qt(t):
            # QT[n, (b2, i)] = sum_k D[k, n] * PT[k, (b2, i)]
            qt_sb = qt_pool.tile([128, 2, 2, N], bf16)  # (ncx, b2, i)
            for ncx in range(2):
                qt_ps = psA_pool.tile([128, 2, N], f32, tag="qtps")
                for kc in range(2):
                    nc.tensor.matmul(
                        qt_ps[:].rearrange("p a b -> p (a b)"),
                        lhsT=d_sb[:, kc, ncx * 128 : (ncx + 1) * 128],
                        rhs=pt_tiles[t][:, kc, :, :].rearrange("p a b -> p (a b)"),
                        start=(kc == 0),
                        stop=(kc == 1),
                    )
                nc.scalar.copy(qt_sb[:, ncx], qt_ps[:])
            qt_tiles[t] = qt_sb

        load_pt(0)
        nc.scalar.dma_start(idx0_sb[:], ei_in[:])
        nc.scalar.dma_start(dat0_sb[:], ed_in[:])
        nc.scalar.dma_start(wdup_sb[:], wd_in[:])
        # hoisted batch-level prep: merge dup weights into dat0 and
        # compute all sum(w) columns in three wide ops
        ws_all = const_pool.tile([128, BPC, CD], f32)
        nc.vector.tensor_reduce(
            out=ws_all[:], in_=wdup_sb[:],
            axis=mybir.AxisListType.X, op=mybir.AluOpType.add,
        )
        nc.vector.tensor_copy(dat0_sb[:, :, CS:], ws_all[:])
        nc.vector.tensor_reduce(
            out=red_sb[:, BPC:], in_=dat0_sb[:],
            axis=mybir.AxisListType.X, op=mybir.AluOpType.add,
        )
        do_qt(0)
        load_pt(1)

        for t in range(NPAIR):
            # software pipeline: QT one pair ahead keeps the PE dense
            if t + 1 < NPAIR:
                do_qt(t + 1)
                if t + 2 < NPAIR:
                    load_pt(t + 2)
            qt_sb = qt_tiles[t]
            pt2 = pt_tiles[t]

            for b2 in range(2):
                b = 2 * t + b2
                # ---- W scatter (dup weights already merged into dat0)
                w_sb = w_pool.tile([128, NE], bf16)
                nc.gpsimd.local_scatter(
                    out_ap=w_sb[:],
                    data_ap=dat0_sb[:, b],
                    idxs_ap=idx0_sb[:, b],
                    channels=128,
                    num_elems=NE,
                    num_idxs=C0,
                )
                # ---- G[(ic), j] = sum_n QT[n, i] * PT[n, j]
                g_ps = psB_pool.tile([128, 2, N], f32, tag="gps")  # (ic, j)
                for ic in range(2):
                    for ncx in range(2):
                        nc.tensor.matmul(
                            g_ps[:, ic, :],
                            lhsT=qt_sb[:, ncx, b2, ic * 128 : (ic + 1) * 128],
                            rhs=pt2[:, ncx, b2, :],
                            start=(ncx == 0),
                            stop=(ncx == 1),
                        )

                # ---- <W, G>: reduce the PREVIOUS batch first (pipelined);
                # the multiply reads G straight from PSUM (no copy)
                flush_pending()
                pending = (w_sb, g_ps, b)

        flush_pending()

        # ---- cross-partition reduce of all partials in one matmul
        red_ps = psum1_pool.tile([1, 2 * BPC], f32, tag="redps")
        nc.tensor.matmul(
            red_ps[:], lhsT=ones_sb[:], rhs=red_sb[:], start=True, stop=True
        )
        fin = const_pool.tile([1, 2 * BPC], f32)
        nc.vector.tensor_copy(fin[:], red_ps[:])
        nc.sync.dma_start(out[:], fin[:])

    if not nc.is_finalized():
        nc.finalize()
    return nc


_NC_CACHE = {}


def _get_nc():
    if "nc" not in _NC_CACHE:
        _NC_CACHE["nc"] = _build_bass()
    return _NC_CACHE["nc"]


def _prep_in_maps(P, d_error, edge_i, edge_j, edge_w):
    P = np.asarray(P, dtype=np.float32)
    d_error = np.asarray(d_error, dtype=np.float32)
    edge_i = np.asarray(edge_i, dtype=np.int64)
    edge_j = np.asarray(edge_j, dtype=np.int64)
    edge_w = np.asarray(edge_w, dtype=np.float32)

    # P^T pairs: pt[t, p, kc, b2, i] = P[2t+b2, i, kc*128+p]
    PT = np.ascontiguousarray(np.transpose(P, (0, 2, 1)))  # [B, N(k), N(i)]
    PT = PT.reshape(B // 2, 2, 2, 128, N).transpose(0, 3, 2, 1, 4)
    PT = np.ascontiguousarray(PT).astype(ml_dtypes.bfloat16)
    D = np.ascontiguousarray(d_error.reshape(2, 128, N).transpose(1, 0, 2))
    D = D.astype(ml_dtypes.bfloat16)

    # scatter plan: edge f = 256*i + j -> partition p = (f>>8)&127,
    # slot m = (f>>15)*256 + (f&255). Exact-duplicate edges share a cell.
    f_all = (edge_i << 8) | edge_j  # [B, E]
    p_all = (f_all >> 8) & 127
    m_all = ((f_all >> 15) << 8) | (f_all & 255)

    idx0 = np.full((B, 128, C0), -1, np.int16)
    dat0 = np.zeros((B, 128, C0), np.float32)
    wdup = np.zeros((B, 128, CD, R), np.float32)

    for b in range(B):
        pb, mb, wb = p_all[b], m_all[b], edge_w[b]
        order = np.lexsort((mb, pb))
        ps, ms, ws = pb[order], mb[order], wb[order]
        first = np.ones(E, bool)
        first[1:] = (ps[1:] != ps[:-1]) | (ms[1:] != ms[:-1])
        fidx = np.flatnonzero(first)
        ccnt = np.diff(np.append(fidx, E))
        cp, cm = ps[fidx], ms[fidx]
        # per-partition sequential numbering of single / dup cells
        single = ccnt == 1
        for pp in np.unique(cp):
            sel = cp == pp
            cells_m = cm[sel]
            cells_n = ccnt[sel]
            cells_start = fidx[sel]
            s_slot = 0
            d_slot = 0
            for cmi, cni, csi in zip(cells_m, cells_n, cells_start):
                if cni == 1:
                    assert s_slot < CS
                    idx0[b, pp, s_slot] = cmi
                    dat0[b, pp, s_slot] = ws[csi]
                    s_slot += 1
                else:
                    assert d_slot < CD and cni <= R
                    idx0[b, pp, CS + d_slot] = cmi
                    wdup[b, pp, d_slot, :cni] = ws[csi : csi + cni]
                    d_slot += 1

    dat0 = dat0.astype(ml_dtypes.bfloat16)
    # -> [p, b, ...]
    idx0 = np.ascontiguousarray(idx0.transpose(1, 0, 2))
    dat0 = np.ascontiguousarray(dat0.transpose(1, 0, 2))
    wdup = np.ascontiguousarray(wdup.transpose(1, 0, 2, 3))

    in_maps = []
    for c in range(NCORES):
        sl = slice(c * BPC, (c + 1) * BPC)
        in_maps.append(
            {
                "pt": np.ascontiguousarray(PT[c * NPAIR : (c + 1) * NPAIR]),
                "derr": D,
                "idx0": np.ascontiguousarray(idx0[:, sl]),
                "dat0": np.ascontiguousarray(dat0[:, sl]),
                "wdup": np.ascontiguousarray(wdup[:, sl]),
            }
        )
    return in_maps


def run(P, d_error, edge_i, edge_j, edge_w, trace=False):
    """Run on 8 cores; returns (loss_scalar, BassKernelResults)."""
    nc = _get_nc()
    in_maps = _prep_in_maps(P, d_error, edge_i, edge_j, edge_w)
    res = run_bass_kernel_spmd(
        nc, in_maps, core_ids=list(range(NCORES)), trace=trace
    )
    # host-side all-reduce: loss = mean_b( sl_b / max(sw_b, 1e-8) )
    acc = 0.0
    for r in res.results:
        part = np.asarray(r["out"], dtype=np.float64).reshape(2 * BPC)
        sl, sw = part[:BPC], part[BPC:]
        acc += float(np.sum(sl / np.maximum(sw, 1e-8)))
    loss = np.float32(acc / B)
    return loss, res


def kernel(P, d_error, edge_i, edge_j, edge_w):
    loss, _ = run(P, d_error, edge_i, edge_j, edge_w, trace=False)
    return np.asarray(loss, dtype=np.float32)


# revision 39
# speedup vs baseline: 1.1260x; 1.0742x over previous
"""ErrorAwareEdgeLoss Trainium2 kernel.

Math: loss = mean_b [ (sum_e w_be * P[b,i_e,:] @ D @ P[b,j_e,:]) / max(sum_e w_be, 1e-8) ]

Reformulation:
    G_b = (P_b @ D) @ P_b^T            (two 256^3 matmuls on the PE, bf16)
    sum_e w_be * cost_e = <W_b, G_b>   with W_b[i,j] = sum of w over edges (i,j)

W_b is built entirely in SBUF with one gpsimd local_scatter per batch:
G_b's element f = 256*i + j lives at partition p = (f>>8)&127, local slot
m = (f>>15)*256 + (f&255) — a bijection, so the host can bin each edge to
its home partition (pure index/layout prep). Exact duplicate edges (same
(i,j)) share a cell; their weights are summed ON DEVICE (a tiny reduce
over a host-laid [cell, copy] array) before the scatter. <W, G> is then
one elementwise multiply + row reduce; no DRAM spill, no gather DMA.

Sharding: data-parallel over batch: 8 NeuronCores x 8 batches. Each core
emits per-sample partial sums (sum w*g and sum w per batch); the host
performs the final divide + mean (the all-reduce of the sharding hint).
"""

from contextlib import ExitStack

import ml_dtypes
import numpy as np

import concourse.bacc as bacc
import concourse.bass as bass
import concourse.mybir as mybir
import concourse.tile as tile
from concourse.bass_utils import run_bass_kernel_spmd

B, N, E = 64, 256, 8192
NCORES = 8
BPC = B // NCORES  # batches per core
NPAIR = BPC // 2

C0 = 104  # scatter slots per partition (singles + dup cells)
CD = 16   # dup-cell slots (tail of the C0 range)
CS = C0 - CD  # single-cell slots
R = 12    # max copies per dup cell

f32 = mybir.dt.float32
bf16 = mybir.dt.bfloat16
i16 = mybir.dt.int16

NE = 512  # W/G elements per partition


def _build_bass():
    nc = bacc.Bacc("TRN2", target_bir_lowering=False, debug=False)

    # pt[t, p, kc, b2, i] = P[2t+b2, i, kc*128+p]
    pt_in = nc.dram_tensor("pt", [NPAIR, 128, 2, 2, N], bf16, kind="ExternalInput")
    d_in = nc.dram_tensor("derr", [128, 2, N], bf16, kind="ExternalInput")
    # per-(batch, partition) scatter plan (see _prep_in_maps)
    ei_in = nc.dram_tensor("idx0", [128, BPC, C0], i16, kind="ExternalInput")
    ed_in = nc.dram_tensor("dat0", [128, BPC, C0], bf16, kind="ExternalInput")
    wd_in = nc.dram_tensor("wdup", [128, BPC, CD, R], bf16, kind="ExternalInput")
    out = nc.dram_tensor("out", [1, 2 * BPC], f32, kind="ExternalOutput")

    with tile.TileContext(nc) as tc, ExitStack() as ctx:
        const_pool = ctx.enter_context(tc.tile_pool(name="const", bufs=1))
        pt_pool = ctx.enter_context(tc.tile_pool(name="pt", bufs=4))
        qt_pool = ctx.enter_context(tc.tile_pool(name="qt", bufs=3))
        g_pool = ctx.enter_context(tc.tile_pool(name="g", bufs=3))
        w_pool = ctx.enter_context(tc.tile_pool(name="w", bufs=4))
        e_pool = ctx.enter_context(tc.tile_pool(name="e", bufs=4))
        psA_pool = ctx.enter_context(tc.tile_pool(name="psA", bufs=4, space="PSUM"))
        psB_pool = ctx.enter_context(tc.tile_pool(name="psB", bufs=3, space="PSUM"))
        psum1_pool = ctx.enter_context(tc.tile_pool(name="ps1", bufs=1, space="PSUM"))

        # inputs (sync queue; d + pt0 first, edge plan next, later pts after)
        d_sb = const_pool.tile([128, 2, N], bf16)
        nc.sync.dma_start(d_sb[:], d_in[:])
        idx0_sb = const_pool.tile([128, BPC, C0], i16)
        dat0_sb = const_pool.tile([128, BPC, C0], bf16)
        wdup_sb = const_pool.tile([128, BPC, CD, R], bf16)
        # per-batch partials: cols [0,BPC) = sum(w*g), cols [BPC,2*BPC) = sum(w)
        red_sb = const_pool.tile([128, 2 * BPC], f32)
        ones_sb = const_pool.tile([128, 1], f32)
        nc.vector.memset(ones_sb[:], 1.0)

        pending = None  # (w_tile, g_psum, b) awaiting <W, G> — one batch behind

        def flush_pending():
            nonlocal pending
            if pending is None:
                return
            w_sb, g_sb, b = pending
            prod = e_pool.tile([128, NE], bf16, tag="prod")
            nc.vector.tensor_tensor(
                out=prod[:],
                in0=w_sb[:],
                in1=g_sb[:].rearrange("p a b -> p (a b)"),
                op=mybir.AluOpType.mult,
            )
            nc.vector.tensor_reduce(
                out=red_sb[:, b : b + 1],
                in_=prod[:],
                axis=mybir.AxisListType.X,
                op=mybir.AluOpType.add,
            )
            pending = None

        pt_tiles = [None] * NPAIR
        qt_tiles = [None] * NPAIR

        def load_pt(t):
            pt2 = pt_pool.tile([128, 2, 2, N], bf16)
            if t == 0:
                # halves on different queues transfer in parallel
                nc.sync.dma_start(pt2[:, 0], pt_in[t, :, 0])
                nc.scalar.dma_start(pt2[:, 1], pt_in[t, :, 1])
            else:
                nc.sync.dma_start(pt2[:], pt_in[t])
            pt_tiles[t] = pt2

        def do_qt(t):
            # QT[n, (b2, i)] = sum_k D[k, n] * PT[k, (b2, i)]
            qt_sb = qt_pool.tile([128, 2, 2, N], bf16)  # (ncx, b2, i)
            for ncx in range(2):
                qt_ps = psA_pool.tile([128, 2, N], f32, tag="qtps")
                for kc in range(2):
                    nc.tensor.matmul(
                        qt_ps[:].rearrange("p a b -> p (a b)"),
                        lhsT=d_sb[:, kc, ncx * 128 : (ncx + 1) * 128],
                        rhs=pt_tiles[t][:, kc, :, :].rearrange("p a b -> p (a b)"),
                        start=(kc == 0),
                        stop=(kc == 1),
                    )
                nc.scalar.copy(qt_sb[:, ncx], qt_ps[:])
            qt_tiles[t] = qt_sb

        load_pt(0)
        load_pt(1)
        nc.sync.dma_start(wdup_sb[:], wd_in[:])
        nc.sync.dma_start(dat0_sb[:], ed_in[:])
        nc.sync.dma_start(idx0_sb[:], ei_in[:])
        # hoisted batch-level prep: merge dup weights into dat0 and
        # compute all sum(w) columns in three wide ops
        ws_all = const_pool.tile([128, BPC, CD], f32)
        nc.vector.tensor_reduce(
            out=ws_all[:], in_=wdup_sb[:],
            axis=mybir.AxisListType.X, op=mybir.AluOpType.add,
        )
        nc.vector.tensor_copy(dat0_sb[:, :, CS:], ws_all[:])
        nc.vector.tensor_reduce(
            out=red_sb[:, BPC:], in_=dat0_sb[:],
            axis=mybir.AxisListType.X, op=mybir.AluOpType.add,
        )
        do_qt(0)

        for t in range(NPAIR):
            # software pipeline: QT one pair ahead keeps the PE dense
            if t + 1 < NPAIR:
                do_qt(t + 1)
                if t + 2 < NPAIR:
                    load_pt(t + 2)
            qt_sb = qt_tiles[t]
            pt2 = pt_tiles[t]

            for b2 in range(2):
                b = 2 * t + b2
                # ---- W scatter (dup weights already merged into dat0)
                w_sb = w_pool.tile([128, NE], bf16)
                nc.gpsimd.local_scatter(
                    out_ap=w_sb[:],
                    data_ap=dat0_sb[:, b],
                    idxs_ap=idx0_sb[:, b],
                    channels=128,
                    num_elems=NE,
                    num_idxs=C0,
                )
                # ---- G[(ic), j] = sum_n QT[n, i] * PT[n, j]
                g_ps = psB_pool.tile([128, 2, N], f32, tag="gps")  # (ic, j)
                for ic in range(2):
                    for ncx in range(2):
                        nc.tensor.matmul(
                            g_ps[:, ic, :],
                            lhsT=qt_sb[:, ncx, b2, ic * 128 : (ic + 1) * 128],
                            rhs=pt2[:, ncx, b2, :],
                            start=(ncx == 0),
                            stop=(ncx == 1),
                        )

                # ---- <W, G>: reduce the PREVIOUS batch first (pipelined);
                # the multiply reads G straight from PSUM (no copy)
                flush_pending()
                pending = (w_sb, g_ps, b)

        flush_pending()

        # ---- cross-partition reduce of all partials in one matmul
        red_ps = psum1_pool.tile([1, 2 * BPC], f32, tag="redps")
        nc.tensor.matmul(
            red_ps[:], lhsT=ones_sb[:], rhs=red_sb[:], start=True, stop=True
        )
        fin = const_pool.tile([1, 2 * BPC], f32)
        nc.vector.tensor_copy(fin[:], red_ps[:])
        nc.sync.dma_start(out[:], fin[:])

    if not nc.is_finalized():
        nc.finalize()
    return nc


_NC_CACHE = {}


def _get_nc():
    if "nc" not in _NC_CACHE:
        _NC_CACHE["nc"] = _build_bass()
    return _NC_CACHE["nc"]


def _prep_in_maps(P, d_error, edge_i, edge_j, edge_w):
    P = np.asarray(P, dtype=np.float32)
    d_error = np.asarray(d_error, dtype=np.float32)
    edge_i = np.asarray(edge_i, dtype=np.int64)
    edge_j = np.asarray(edge_j, dtype=np.int64)
    edge_w = np.asarray(edge_w, dtype=np.float32)

    # P^T pairs: pt[t, p, kc, b2, i] = P[2t+b2, i, kc*128+p]
    PT = np.ascontiguousarray(np.transpose(P, (0, 2, 1)))  # [B, N(k), N(i)]
    PT = PT.reshape(B // 2, 2, 2, 128, N).transpose(0, 3, 2, 1, 4)
    PT = np.ascontiguousarray(PT).astype(ml_dtypes.bfloat16)
    D = np.ascontiguousarray(d_error.reshape(2, 128, N).transpose(1, 0, 2))
    D = D.astype(ml_dtypes.bfloat16)

    # scatter plan: edge f = 256*i + j -> partition p = (f>>8)&127,
    # slot m = (f>>15)*256 + (f&255). Exact-duplicate edges share a cell.
    f_all = (edge_i << 8) | edge_j  # [B, E]
    p_all = (f_all >> 8) & 127
    m_all = ((f_all >> 15) << 8) | (f_all & 255)

    idx0 = np.full((B, 128, C0), -1, np.int16)
    dat0 = np.zeros((B, 128, C0), np.float32)
    wdup = np.zeros((B, 128, CD, R), np.float32)  # cast to bf16 after fill

    for b in range(B):
        pb, mb, wb = p_all[b], m_all[b], edge_w[b]
        order = np.lexsort((mb, pb))
        ps, ms, ws = pb[order], mb[order], wb[order]
        first = np.ones(E, bool)
        first[1:] = (ps[1:] != ps[:-1]) | (ms[1:] != ms[:-1])
        fidx = np.flatnonzero(first)
        ccnt = np.diff(np.append(fidx, E))
        cp, cm = ps[fidx], ms[fidx]
        # per-partition sequential numbering of single / dup cells
        single = ccnt == 1
        for pp in np.unique(cp):
            sel = cp == pp
            cells_m = cm[sel]
            cells_n = ccnt[sel]
            cells_start = fidx[sel]
            s_slot = 0
            d_slot = 0
            for cmi, cni, csi in zip(cells_m, cells_n, cells_start):
                if cni == 1:
                    assert s_slot < CS
                    idx0[b, pp, s_slot] = cmi
                    dat0[b, pp, s_slot] = ws[csi]
                    s_slot += 1
                else:
                    assert d_slot < CD and cni <= R
                    idx0[b, pp, CS + d_slot] = cmi
                    wdup[b, pp, d_slot, :cni] = ws[csi : csi + cni]
                    d_slot += 1

    dat0 = dat0.astype(ml_dtypes.bfloat16)
    wdup = wdup.astype(ml_dtypes.bfloat16)
    # -> [p, b, ...]
    idx0 = np.ascontiguousarray(idx0.transpose(1, 0, 2))
    dat0 = np.ascontiguousarray(dat0.transpose(1, 0, 2))
    wdup = np.ascontiguousarray(wdup.transpose(1, 0, 2, 3))

    in_maps = []
    for c in range(NCORES):
        sl = slice(c * BPC, (c + 1) * BPC)
        in_maps.append(
            {
                "pt": np.ascontiguousarray(PT[c * NPAIR : (c + 1) * NPAIR]),
                "derr": D,
                "idx0": np.ascontiguousarray(idx0[:, sl]),
                "dat0": np.ascontiguousarray(dat0[:, sl]),
                "wdup": np.ascontiguousarray(wdup[:, sl]),
            }
        )
    return in_maps


def run(P, d_error, edge_i, edge_j, edge_w, trace=False):
    """Run on 8 cores; returns (loss_scalar, BassKernelResults)."""
    nc = _get_nc()
    in_maps = _prep_in_maps(P, d_error, edge_i, edge_j, edge_w)
    res = run_bass_kernel_spmd(
        nc, in_maps, core_ids=list(range(NCORES)), trace=trace
    )
    # host-side all-reduce: loss = mean_b( sl_b / max(sw_b, 1e-8) )
    acc = 0.0
    for r in res.results:
        part = np.asarray(r["out"], dtype=np.float64).reshape(2 * BPC)
        sl, sw = part[:BPC], part[BPC:]
        acc += float(np.sum(sl / np.maximum(sw, 1e-8)))
    loss = np.float32(acc / B)
    return loss, res


def kernel(P, d_error, edge_i, edge_j, edge_w):
    loss, _ = run(P, d_error, edge_i, edge_j, edge_w, trace=False)
    return np.asarray(loss, dtype=np.float32)
